# revision 5
# baseline (speedup 1.0000x reference)
"""Trainium2 8-core kernel for nn_AdvancedEmbeddingBlock — v3.

Reference:
    x_phys = props[x] @ phys_w + phys_b
    x_ = concat(emb_w[x], tag_w[tag], x_phys, period_w[period_idx[x]], group_w[group_idx[x]])
    rbf_h = silu(rbf @ rbf_w + rbf_b)
    out = silu(concat(x_[i], x_[j], rbf_h) @ lin_w + lin_b)        # [E, 256]

Rewrites:
    out = silu(A[i] + B[j] + rbf_h @ W3 + lin_b); the A/B lookups become
    one-hot matmuls over 176 mask rows (85 x_i + 3 tag_i + 3 tag_j + 85 x_j).
    silu(y) = 0.5*y + g(y), g(y) = 0.5*y*tanh(y/2):
      - the linear 0.5*y*W3 part rides an exact bf16 GEMM M2 = 0.5*rbf_w@W3
        contracted directly against rbf (no activation on its path)
      - g is evaluated by ONE custom Vector-engine op as an even deg-8
        polynomial (coefficients fitted end-to-end), written in fp8e4
      - W3^T @ g runs as an fp8 DoubleRow matmul: virtual K=256 in a single
        MM per 128-row output half (2x contraction throughput)
    Per 512-edge subtile the PE does 6 full-mode bf16/fp8 matmuls + 2
    DoubleRow matmuls (vs 10 in the lookup-only baseline), ScalarE does one
    silu pass (the inner activation moved to the Vector engine), and
    subtiles are processed in two-subtile phases (all GEMMs, then all
    DoubleRow) to minimize PE array mode switches.
"""

import numpy as np
import ml_dtypes

import concourse.bass as bass  # noqa: F401
import concourse.mybir as mybir
import concourse.tile as tile
from concourse import bacc
from concourse.bass_utils import run_bass_kernel_spmd

BF16 = mybir.dt.bfloat16
FP8 = mybir.dt.float8e4
FP32 = mybir.dt.float32
AFT = mybir.ActivationFunctionType
DR = mybir.MatmulPerfMode.DoubleRow

N_CORES = 8
N_EDGES = 400000
HID = 256
ET = 512                      # edges per compute subtile
MACRO = 7168                  # edges per DMA macro-tile (= 14 * 512)
NSUB = MACRO // ET
E_LOC = 50176                 # padded edges per core (= 7 * 7168)
NM = E_LOC // MACRO
E_PAD = N_CORES * E_LOC       # 401408

# g(y0) = silu(y0) - 0.5*y0 ~= c1*r + c2*r^2 + c3*r^3 + c4*r^4, r = (0.5*y0)^2
POLY_C = (9.50526108e-01, -1.97011201e-01, 2.23990092e-02, -8.41269962e-04)
# The DVE computes rh*(C1 + rh*(C2 + rh*(C3 + rh))) with rh = (kappa*y)^2 and
# leading coefficient +1; c4 < 0 is absorbed by negating the output (W3 is
# negated on the host) and scaling the input by kappa (folded into RW').
KAPPA = float((-POLY_C[3]) ** 0.125)
C1_DVE = float(-POLY_C[0] / KAPPA**2)
C2_DVE = float(-POLY_C[1] / KAPPA**4)
C3_DVE = float(-POLY_C[2] / KAPPA**6)

_CACHE = {}

DVE_OP_NAME = "GNN_GPOLY_ANT"


def _register_dve():
    """Register the custom DVE op: out = r*(C1 + r*(C2 + r*(C3 + r))),
    r = (Src0 + C0)^2; C0/C1 [P,1] APs, C2 literal, C3 via Src1 latch."""
    import concourse.dve_ops as dvo
    from concourse.dve_spec import (
        C0, C1, C2, C3, Spec, Src0, lower, sq, _spill_c3_to_src1,
    )
    from concourse.dve_uop import DveOpSpec
    from concourse.dve_ops import DveOp

    if DVE_OP_NAME in dvo._SUB_OPCODE_FOR_NAME:
        return next(op for op in dvo.OPS if op.name == DVE_OP_NAME)

    y = Src0 + C0
    r = sq(y)
    g = ((((C3 + r) * r) + C2) * r + C1) * r
    body = _spill_c3_to_src1(g)

    def ref(in0, in1, s0, s1, imm2):
        yv = in0.astype(np.float32) + np.asarray(s0, np.float32)
        rv = yv * yv
        c3 = np.asarray(in1, np.float32).reshape(in0.shape[0], 1)
        return (rv * (s1 + rv * (imm2 + rv * (c3 + rv)))).astype(np.float32)

    spec = Spec(body=body, reference=ref)
    row = max(dvo._SUB_OPCODE_FOR_NAME.values()) + 1
    assert row < 0x20
    dvo._SUB_OPCODE_FOR_NAME[DVE_OP_NAME] = row
    shas = {}
    for ver in ("v3", "v4"):
        s = DveOpSpec(name=DVE_OP_NAME, opcode=row, uops=lower(spec, ver=ver),
                      rd1_en=True)
        shas[ver] = s.sha(ver)
    op = DveOp(DVE_OP_NAME, spec, subdim=False, uops_sha=shas)
    dvo.OPS.append(op)
    dvo.CUSTOM_DVE_SPECS[DVE_OP_NAME] = spec
    return op


def _build_nc():
    gp = _register_dve()
    nc = bacc.Bacc("TRN2", target_bir_lowering=False, debug=False,
                   enable_asserts=False, num_devices=N_CORES)
    # mbr: rows 0-63 rbf^T (bf16), rows 64-111 leftover x_j one-hot (bf16)
    mbr = nc.dram_tensor("mbr", [113, E_LOC], BF16, kind="ExternalInput")
    mhA = nc.dram_tensor("mhA", [128, E_LOC], FP8, kind="ExternalInput")
    cpk = nc.dram_tensor("cpk", [128, 6 * 128], BF16, kind="ExternalInput")
    w3pk = nc.dram_tensor("w3pk", [128, 512], FP8, kind="ExternalInput")
    dvc = nc.dram_tensor("dvc", [128, 3], FP32, kind="ExternalInput")
    outT = nc.dram_tensor("outT", [128, 2 * E_LOC], BF16, kind="ExternalOutput")

    with tile.TileContext(nc) as tc:
        with (
            tc.tile_pool(name="consts", bufs=1) as consts,
            tc.tile_pool(name="io", bufs=3) as io,
            tc.tile_pool(name="om", bufs=3) as om,
            tc.tile_pool(name="mid", bufs=4) as mid,
            tc.tile_pool(name="psum", bufs=1, space="PSUM") as psum,
        ):
            ct = consts.tile([128, 6 * 128], BF16)
            nc.sync.dma_start(ct[:], cpk[:])
            w3t = consts.tile([128, 512], FP8)
            dvt = consts.tile([128, 3], FP32)

            # warm the Silu act table off the critical path
            scr = consts.tile([128, 8], BF16)
            nc.vector.memset(scr[:], 0.0)
            scr2 = consts.tile([128, 8], BF16)
            nc.scalar.activation(scr2[:], scr[:], AFT.Silu)

            rw_f0 = ct[0:113, 0:128]
            rw_f1 = ct[0:113, 128:256]
            mw_f0 = ct[0:112, 256:384]     # [M2(64); WB(48)]
            mw_f1 = ct[0:112, 384:512]
            wa_f0 = ct[:, 512:640]
            wa_f1 = ct[:, 640:768]
            w3_f0 = w3t[:, 0:256].rearrange("p (two m) -> p two m", two=2)
            w3_f1 = w3t[:, 256:512].rearrange("p (two m) -> p two m", two=2)
            c3ap = dvt[:, 2:3]

            for m in range(NM):
                ms = slice(m * MACRO, (m + 1) * MACRO)
                mbr_m = io.tile([128, MACRO], BF16, tag="mbr")
                mhA_m = io.tile([128, MACRO], FP8, tag="mhA")
                if m == 0:
                    first = True
                    for cs in (slice(0, ET), slice(ET, 2 * ET),
                               slice(2 * ET, 4 * ET), slice(4 * ET, MACRO)):
                        nc.sync.dma_start(mbr_m[0:112, cs], mbr[0:112, cs])
                        nc.sync.dma_start(mbr_m[112:113, cs], mbr[112:113, cs])
                        nc.sync.dma_start(mhA_m[:, cs], mhA[:, cs])
                        if first:
                            # deferred consts: not needed until the first
                            # DVE / DoubleRow, well after the first GEMMs
                            nc.sync.dma_start(w3t[:], w3pk[:])
                            nc.sync.dma_start(dvt[:], dvc[:])
                            first = False
                else:
                    nc.sync.dma_start(mbr_m[0:112, :], mbr[0:112, ms])
                    nc.sync.dma_start(mbr_m[112:113, :], mbr[112:113, ms])
                    nc.sync.dma_start(mhA_m[:], mhA[:, ms])
                out_m = om.tile([128, 2 * MACRO], BF16, tag="out_m")

                def front(s):
                    es = slice(s * ET, (s + 1) * ET)
                    p1 = psum.tile([128, 2 * ET], FP32, tag=f"p1{s % 2}")
                    pa = psum.tile([128, ET], FP32, tag=f"p2a{s % 2}")
                    pb = psum.tile([128, ET], FP32, tag=f"p2b{s % 2}")
                    nc.tensor.matmul(p1[:, 0:ET], rw_f0, mbr_m[0:113, es],
                                     start=True, stop=True)
                    nc.tensor.matmul(p1[:, ET:2 * ET], rw_f1, mbr_m[0:113, es],
                                     start=True, stop=True)
                    nc.tensor.matmul(pa, mw_f0, mbr_m[0:112, es],
                                     start=True, stop=False)
                    nc.tensor.matmul(pb, mw_f1, mbr_m[0:112, es],
                                     start=True, stop=False)
                    nc.tensor.matmul(pa, wa_f0, mhA_m[:, es],
                                     start=False, stop=False)
                    nc.tensor.matmul(pb, wa_f1, mhA_m[:, es],
                                     start=False, stop=False)
                    return p1, (pa, pb)

                def dve(s, p1):
                    g8 = mid.tile([128, 2 * ET], FP8, tag=f"g8{s % 2}")
                    nc.vector._custom_dve(gp, out=g8[:], in0=p1[:],
                                          in1=c3ap, s0=0.0, s1=C1_DVE,
                                          imm2=C2_DVE)
                    return g8

                def back(s, p2, g8):
                    pa, pb = p2
                    g2 = g8[:].rearrange("p (two n) -> p two n", two=2)
                    nc.tensor.matmul(pa[:], w3_f0, g2, start=False,
                                     stop=True, perf_mode=DR)
                    nc.tensor.matmul(pb[:], w3_f1, g2, start=False,
                                     stop=True, perf_mode=DR)
                    # split silu: pa's act fires after DR-f0 and frees its
                    # PSUM bank one DR + one act earlier
                    nc.scalar.activation(out_m[:, 2 * s * ET:(2 * s + 1) * ET],
                                         pa[:], AFT.Silu)
                    nc.scalar.activation(out_m[:, (2 * s + 1) * ET:2 * (s + 1) * ET],
                                         pb[:], AFT.Silu)

                # two-subtile phases: all GEMMs, then both DoubleRow pairs
                for sp in range(0, NSUB, 2):
                    p1a_, p2a_ = front(sp)
                    ga = dve(sp, p1a_)
                    p1b_, p2b_ = front(sp + 1)
                    gb = dve(sp + 1, p1b_)
                    back(sp, p2a_, ga)
                    back(sp + 1, p2b_, gb)

                # output stores ride the GpSimd DMA queue so input loads
                # never queue behind them on the Sync engine
                if m == NM - 1:
                    for s in range(NSUB):
                        cs = slice(2 * (m * MACRO + s * ET),
                                   2 * (m * MACRO + (s + 1) * ET))
                        nc.gpsimd.dma_start(outT[:, cs],
                                            out_m[:, 2 * s * ET:2 * (s + 1) * ET])
                else:
                    nc.gpsimd.dma_start(outT[:, 2 * m * MACRO:2 * (m + 1) * MACRO],
                                        out_m[:])
    nc.compile()
    return nc


def get_nc():
    if "nc" not in _CACHE:
        _CACHE["nc"] = _build_nc()
    return _CACHE["nc"]


def prepare(x, tag, i, j, rbf, period_idx, group_idx, props,
            emb_w, tag_w, period_w, group_w, phys_w, phys_b,
            rbf_w, rbf_b, lin_w, lin_b):
    bf = ml_dtypes.bfloat16
    f8n = ml_dtypes.float8_e4m3fn
    f32 = np.float32
    x = np.asarray(x).astype(np.int64)
    tag = np.asarray(tag).astype(np.int64)
    i = np.asarray(i).astype(np.int64)
    j = np.asarray(j).astype(np.int64)
    rbf = np.asarray(rbf, dtype=f32)
    period_idx = np.asarray(period_idx).astype(np.int64)
    group_idx = np.asarray(group_idx).astype(np.int64)
    props, emb_w, tag_w = (np.asarray(v, dtype=f32) for v in (props, emb_w, tag_w))
    period_w, group_w = np.asarray(period_w, dtype=f32), np.asarray(group_w, dtype=f32)
    phys_w, phys_b = np.asarray(phys_w, dtype=f32), np.asarray(phys_b, dtype=f32)
    rbf_w, rbf_b = np.asarray(rbf_w, dtype=f32), np.asarray(rbf_b, dtype=f32)
    lin_w, lin_b = np.asarray(lin_w, dtype=f32), np.asarray(lin_b, dtype=f32)

    x_phys = props @ phys_w + phys_b
    U85 = np.zeros((85, HID), f32)
    U85[:, 0:128] = emb_w
    U85[:, 160:192] = x_phys
    U85[:, 192:224] = period_w[period_idx]
    U85[:, 224:256] = group_w[group_idx]
    Utag = np.zeros((3, HID), f32)
    Utag[:, 128:160] = tag_w
    W1 = lin_w[0:256]
    W2 = lin_w[256:512]
    W3 = lin_w[512:768]
    AU = U85 @ W1 + lin_b + 0.5 * (rbf_b @ W3)   # linear-part bias folded in
    AT = Utag @ W1
    BU = U85 @ W2
    BT = Utag @ W2
    T2 = np.concatenate([AU, AT, BT], 0)          # [91, 256]
    T3 = BU                                       # [85, 256]
    WA = np.concatenate([T2, T3[0:37]], 0)        # [128, 256]
    WB = T3[37:85]                                # [48, 256]
    RWk = (0.5 * KAPPA) * rbf_w                   # [64, 256]
    M2 = 0.5 * (rbf_w @ W3)                       # [64, 256]
    MW = np.concatenate([M2, WB], 0)              # [112, 256]

    CPK = np.zeros((128, 6 * 128), f32)
    CPK[0:64, 0:128] = RWk[:, 0:128]
    CPK[0:64, 128:256] = RWk[:, 128:256]
    CPK[112, 0:128] = (0.5 * KAPPA) * rbf_b[0:128]
    CPK[112, 128:256] = (0.5 * KAPPA) * rbf_b[128:256]
    CPK[0:112, 256:384] = MW[:, 0:128]
    CPK[0:112, 384:512] = MW[:, 128:256]
    CPK[:, 512:640] = WA[:, 0:128]
    CPK[:, 640:768] = WA[:, 128:256]
    CPK = CPK.astype(bf)

    W3n = (-W3).astype(f8n)                       # output-negation absorb
    W3PK = np.zeros((128, 512), f8n)
    W3PK[:, 0:128] = W3n[0:128, 0:128]
    W3PK[:, 128:256] = W3n[128:256, 0:128]
    W3PK[:, 256:384] = W3n[0:128, 128:256]
    W3PK[:, 384:512] = W3n[128:256, 128:256]

    DVC = np.zeros((128, 3), f32)
    DVC[:, 0] = (0.5 * KAPPA) * rbf_b[0:128]
    DVC[:, 1] = (0.5 * KAPPA) * rbf_b[128:256]
    DVC[:, 2] = C3_DVE

    ONE8 = np.uint8(0x38)                         # 1.0 in e4m3
    ONE16 = np.uint16(0x3F80)                     # 1.0 in bf16
    ar = np.arange(N_EDGES)
    xi, ti_ = x[i], tag[i]
    xj, tj = x[j], tag[j]
    mhA = np.zeros((128, E_PAD), np.uint8)
    mhA[xi, ar] = ONE8
    mhA[85 + ti_, ar] = ONE8
    mhA[88 + tj, ar] = ONE8
    lo = xj < 37
    mhA[91 + xj[lo], ar[lo]] = ONE8
    mhA = mhA.view(ml_dtypes.float8_e4m3fn)

    MBR = np.zeros((113, E_PAD), np.uint16)
    MBR[0:64, 0:N_EDGES] = rbf.astype(bf).T.view(np.uint16)
    hi = ~lo
    MBR[64 + (xj[hi] - 37), ar[hi]] = ONE16
    MBR[112, :] = ONE16
    MBR = MBR.view(bf)

    in_maps = []
    for c in range(N_CORES):
        sl = slice(c * E_LOC, (c + 1) * E_LOC)
        in_maps.append(dict(
            mbr=np.ascontiguousarray(MBR[:, sl]),
            mhA=np.ascontiguousarray(mhA[:, sl]),
            cpk=CPK, w3pk=W3PK, dvc=DVC,
        ))
    return in_maps


def unshard(results):
    out = np.empty((N_EDGES, HID), np.float32)
    for c in range(N_CORES):
        lo = c * E_LOC
        hi = min(lo + E_LOC, N_EDGES)
        n = hi - lo
        arr = np.asarray(results[c]["outT"]).reshape(128, NM * NSUB, 2, ET)
        f0 = arr[:, :, 0, :].transpose(1, 2, 0).reshape(E_LOC, 128)
        f1 = arr[:, :, 1, :].transpose(1, 2, 0).reshape(E_LOC, 128)
        out[lo:hi, 0:128] = f0[:n].astype(np.float32)
        out[lo:hi, 128:256] = f1[:n].astype(np.float32)
    return out


def kernel(**inputs):
    in_maps = prepare(**inputs)
    nc = get_nc()
    res = run_bass_kernel_spmd(nc, in_maps, core_ids=list(range(N_CORES)))
    return unshard(res.results)
